# revision 1
# baseline (speedup 1.0000x reference)
"""Trainium2 Bass kernel for nn_CRModule (retrieval_knn).

reference:
    xf = x.reshape(4096, 4096); xa = xf[:, ::2]; xb = xf[:, 1::2]   # [T=4096, 2048]
    sq[i,j] = |xa[:,i]|^2 + |xb[:,j]|^2 - 2 * xa[:,i].xb[:,j]
    xdist = sqrt(max(sq, 0))
    wsum = fc_weight.sum(0); wa = wsum[::2]; wb = wsum[1::2]
    scores[i,j] = ((wa[i] + wb[j]) * xdist[i,j])**2
                = (wa[i] + wb[j])**2 * max(sq[i,j], 0)     # sqrt cancels

Strategy (8 NeuronCores):
  Launch 1 (sharded reductions):
    - fc_weight row-sharded (1536 rows/core): partial column sums via
      ones[128,1].T @ fc_tile matmuls -> wpart[1, 4096] per core.
    - column norms na/nb sharded over the 2048 'a'/'b' channels
      (256/core): square on ScalarE, ones-matmul column-sum on PE.
    Host sums the 8 wpart vectors and concatenates na/nb slices (32 KB).
  Launch 2 (main, row-sharded output):
    - each core owns 256 rows of scores; computes (-2a)^T b in bf16 on PE
      (256 matmuls of [128,128]x[128,512], K accumulated in PSUM),
      adds na/nb (fp32, fused DVE op), multiplies by (wa+wb)^2 (fp32).
"""

import numpy as np
import ml_dtypes

import concourse.bass as bass
import concourse.tile as tile
from concourse import bacc, mybir
from concourse.bass_utils import run_bass_kernel_spmd

BF16 = mybir.dt.bfloat16
F32 = mybir.dt.float32
NP_BF16 = ml_dtypes.bfloat16

D = 8          # cores
T = 4096       # inner (contraction) dim = B*N
KT = T // 128  # 32 k-tiles
CA = 2048      # C/2 channels
MLOC = CA // D  # 256 output rows per core
O = 12288      # fc rows
OLOC = O // D   # 1536 fc rows per core
OT = OLOC // 128  # 12 o-tiles per core
C = 4096

_cache = {}


def _new_nc():
    return bacc.Bacc("TRN2", target_bir_lowering=False, debug=False, num_devices=D)


def _build_phase1():
    """Per-core: partial fc column-sum + sharded xa/xb column sq-norms."""
    nc = _new_nc()
    fc_d = nc.dram_tensor("fc", [128, OT, C], BF16, kind="ExternalInput").ap()
    xas_d = nc.dram_tensor("xas", [128, KT, MLOC], BF16, kind="ExternalInput").ap()
    xbs_d = nc.dram_tensor("xbs", [128, KT, MLOC], BF16, kind="ExternalInput").ap()
    wpart_d = nc.dram_tensor("wpart", [1, C], F32, kind="ExternalOutput").ap()
    nasl_d = nc.dram_tensor("nasl", [1, MLOC], F32, kind="ExternalOutput").ap()
    nbsl_d = nc.dram_tensor("nbsl", [1, MLOC], F32, kind="ExternalOutput").ap()

    with tile.TileContext(nc) as tc:
        with (
            tc.tile_pool(name="big", bufs=1) as big,
            tc.tile_pool(name="small", bufs=1) as small,
            tc.tile_pool(name="stage", bufs=2) as stage,
            tc.tile_pool(name="psw", bufs=2, space="PSUM") as psw,
            tc.tile_pool(name="psn", bufs=2, space="PSUM") as psn,
        ):
            ones = small.tile([128, 1], BF16)
            nc.vector.memset(ones[:], 1.0)

            fc_sb = big.tile([128, OT, C], BF16)
            nc.sync.dma_start(fc_sb[:], fc_d[:])

            xas_sb = big.tile([128, KT, MLOC], BF16)
            nc.sync.dma_start(xas_sb[:], xas_d[:])
            xbs_sb = big.tile([128, KT, MLOC], BF16)
            nc.sync.dma_start(xbs_sb[:], xbs_d[:])

            # squares (ScalarE)
            xa2 = big.tile([128, KT, MLOC], BF16)
            nc.scalar.square(xa2[:], xas_sb[:])
            xb2 = big.tile([128, KT, MLOC], BF16)
            nc.scalar.square(xb2[:], xbs_sb[:])

            # partial fc column sums: for each 512-col chunk accumulate
            # over the 12 o-tiles
            wsb = big.tile([1, C], F32)
            for ch in range(C // 512):
                ps = psw.tile([1, 512], F32)
                for ot in range(OT):
                    nc.tensor.matmul(
                        ps[:],
                        ones[:],
                        fc_sb[:, ot, ch * 512:(ch + 1) * 512],
                        start=(ot == 0),
                        stop=(ot == OT - 1),
                    )
                nc.vector.tensor_copy(wsb[:, ch * 512:(ch + 1) * 512], ps[:])
            nc.sync.dma_start(wpart_d[:], wsb[:])

            # na / nb slices: accumulate over 32 k-tiles
            for (x2, out_d) in ((xa2, nasl_d), (xb2, nbsl_d)):
                ps = psn.tile([1, MLOC], F32)
                for kt in range(KT):
                    nc.tensor.matmul(
                        ps[:],
                        ones[:],
                        x2[:, kt, :],
                        start=(kt == 0),
                        stop=(kt == KT - 1),
                    )
                st = stage.tile([1, MLOC], F32)
                nc.vector.tensor_copy(st[:], ps[:])
                nc.sync.dma_start(out_d[:], st[:])

    nc.compile()
    return nc


def _build_phase2():
    """Per-core: 256 rows of scores = (wa+wb)^2 * relu(na+nb-2ab)."""
    nc = _new_nc()
    xasc_d = nc.dram_tensor("xasc", [128, KT, MLOC], BF16, kind="ExternalInput").ap()
    xbr_d = nc.dram_tensor("xbr", [128, KT, CA], BF16, kind="ExternalInput").ap()
    wa2_d = nc.dram_tensor("wa2", [128, 2], F32, kind="ExternalInput").ap()
    nav_d = nc.dram_tensor("nav", [128, 2], F32, kind="ExternalInput").ap()
    wbv_d = nc.dram_tensor("wbv", [1, CA], F32, kind="ExternalInput").ap()
    nbv_d = nc.dram_tensor("nbv", [1, CA], F32, kind="ExternalInput").ap()
    out_d = nc.dram_tensor("scores", [MLOC, CA], F32, kind="ExternalOutput").ap()

    NJ = CA // 512  # 4 column chunks
    MT = MLOC // 128  # 2 m-tiles

    with tile.TileContext(nc) as tc:
        with (
            tc.tile_pool(name="big", bufs=1) as big,
            tc.tile_pool(name="small", bufs=1) as small,
            tc.tile_pool(name="w2p", bufs=1) as w2p,
            tc.tile_pool(name="sqp", bufs=3) as sqp,
            tc.tile_pool(name="outp", bufs=3) as outp,
            tc.tile_pool(name="psmm", bufs=4, space="PSUM") as psmm,
        ):
            xasc = big.tile([128, KT, MLOC], BF16)
            nc.sync.dma_start(xasc[:], xasc_d[:])
            xbr = big.tile([128, KT, CA], BF16)
            nc.sync.dma_start(xbr[:], xbr_d[:])

            wa2 = small.tile([128, 2], F32)
            nc.sync.dma_start(wa2[:], wa2_d[:])
            nav = small.tile([128, 2], F32)
            nc.sync.dma_start(nav[:], nav_d[:])

            # broadcast wb / nb across partitions via DMA
            wb_bc = small.tile([128, CA], F32)
            nc.sync.dma_start(wb_bc[:], wbv_d.to_broadcast([128, CA]))
            nb_bc = small.tile([128, CA], F32)
            nc.sync.dma_start(nb_bc[:], nbv_d.to_broadcast([128, CA]))

            # W2[m][p, j] = (wa[m*128+p] + wb[j])^2
            w2 = []
            for m in range(MT):
                w2m = w2p.tile([128, CA], F32, tag=f"w2_{m}")
                nc.scalar.activation(
                    w2m[:], wb_bc[:],
                    mybir.ActivationFunctionType.Square,
                    bias=wa2[:, m:m + 1], scale=1.0,
                )
                w2.append(w2m)

            for m in range(MT):
                for nj in range(NJ):
                    ps = psmm.tile([128, 512], F32)
                    for kt in range(KT):
                        nc.tensor.matmul(
                            ps[:],
                            xasc[:, kt, m * 128:(m + 1) * 128],
                            xbr[:, kt, nj * 512:(nj + 1) * 512],
                            start=(kt == 0),
                            stop=(kt == KT - 1),
                        )
                    # sq = ps + na[m-part] + nb[chunk]
                    sq = sqp.tile([128, 512], F32)
                    nc.vector.scalar_tensor_tensor(
                        sq[:], ps[:], nav[:, m:m + 1],
                        nb_bc[:, nj * 512:(nj + 1) * 512],
                        op0=mybir.AluOpType.add, op1=mybir.AluOpType.add,
                    )
                    # out = relu(sq) * W2
                    ot = outp.tile([128, 512], F32)
                    nc.vector.scalar_tensor_tensor(
                        ot[:], sq[:], 0.0,
                        w2[m][:, nj * 512:(nj + 1) * 512],
                        op0=mybir.AluOpType.max, op1=mybir.AluOpType.mult,
                    )
                    nc.sync.dma_start(
                        out_d[m * 128:(m + 1) * 128, nj * 512:(nj + 1) * 512],
                        ot[:],
                    )

    nc.compile()
    return nc


def _p_major(a, np_dtype):
    """[n*128, cols] -> [128, n, cols] with tile index in the middle."""
    n = a.shape[0] // 128
    return np.ascontiguousarray(
        a.reshape(n, 128, a.shape[1]).transpose(1, 0, 2)
    ).astype(np_dtype)


def kernel(x, fc_weight, _trace=False):
    x = np.asarray(x, dtype=np.float32)
    fc = np.asarray(fc_weight, dtype=np.float32)

    xf = x.reshape(T, C)
    xa = np.ascontiguousarray(xf[:, 0::2])   # [T, CA]
    xb = np.ascontiguousarray(xf[:, 1::2])

    xb_r = _p_major(xb, NP_BF16)             # [128, KT, CA]
    xa_bf = xa.astype(NP_BF16)
    xa_s2 = (-2.0 * xa).astype(NP_BF16)

    # ---- launch 1 ----
    if "p1" not in _cache:
        _cache["p1"] = _build_phase1()
    nc1 = _cache["p1"]

    in_maps1 = []
    for d in range(D):
        fc_d = _p_major(fc[d * OLOC:(d + 1) * OLOC], NP_BF16)  # [128, OT, C]
        sl = slice(d * MLOC, (d + 1) * MLOC)
        in_maps1.append({
            "fc": fc_d,
            "xas": _p_major(xa_bf[:, sl].astype(NP_BF16), NP_BF16),
            "xbs": _p_major(xb[:, sl], NP_BF16),
        })
    res1 = run_bass_kernel_spmd(nc1, in_maps1, core_ids=list(range(D)), trace=_trace)
    t1 = res1.exec_time_ns

    wsum = np.sum([res1.results[d]["wpart"][0] for d in range(D)], axis=0,
                  dtype=np.float32)                              # [C]
    na = np.concatenate([res1.results[d]["nasl"][0] for d in range(D)])  # [CA]
    nb = np.concatenate([res1.results[d]["nbsl"][0] for d in range(D)])
    wa = np.ascontiguousarray(wsum[0::2])
    wb = np.ascontiguousarray(wsum[1::2])

    # ---- launch 2 ----
    if "p2" not in _cache:
        _cache["p2"] = _build_phase2()
    nc2 = _cache["p2"]

    wbv = wb.reshape(1, CA).astype(np.float32)
    nbv = nb.reshape(1, CA).astype(np.float32)
    in_maps2 = []
    for d in range(D):
        sl = slice(d * MLOC, (d + 1) * MLOC)
        in_maps2.append({
            "xasc": _p_major(xa_s2[:, sl].astype(NP_BF16), NP_BF16),
            "xbr": xb_r,
            "wa2": np.ascontiguousarray(wa[sl].reshape(2, 128).T).astype(np.float32),
            "nav": np.ascontiguousarray(na[sl].reshape(2, 128).T).astype(np.float32),
            "wbv": wbv,
            "nbv": nbv,
        })
    res2 = run_bass_kernel_spmd(nc2, in_maps2, core_ids=list(range(D)), trace=_trace)
    t2 = res2.exec_time_ns

    out = np.concatenate([res2.results[d]["scores"] for d in range(D)], axis=0)
    if _trace:
        kernel.last_times = (t1, t2)
    return out.astype(np.float32)


# revision 3
# speedup vs baseline: 1.1734x; 1.1734x over previous
"""Trainium2 Bass kernel for nn_CRModule (retrieval_knn).

reference:
    xf = x.reshape(4096, 4096); xa = xf[:, ::2]; xb = xf[:, 1::2]   # [T=4096, 2048]
    sq[i,j] = |xa[:,i]|^2 + |xb[:,j]|^2 - 2 * xa[:,i].xb[:,j]
    wsum = fc_weight.sum(0); wa = wsum[::2]; wb = wsum[1::2]
    scores[i,j] = ((wa[i] + wb[j]) * sqrt(max(sq,0)))**2
                = (wa[i] + wb[j])**2 * max(sq[i,j], 0)     # sqrt cancels

Strategy (8 NeuronCores, two SPMD launches):
  Launch 1 (sharded reductions):  fc_weight row-sharded -> partial column
    sums; xa/xb column norms sharded over channels. Host combines 32 KB.
  Launch 2 (main, row-sharded output): each core owns 256 rows of scores;
    (-2a)^T b in bf16 on PE with k-OUTER accumulation into all 8 PSUM
    banks so matmuls chase the chunked xb DMA stream; fused DVE epilogue.
"""

import numpy as np
import ml_dtypes

import concourse.bass as bass
import concourse.tile as tile
from concourse import bacc, mybir
from concourse.bass_utils import run_bass_kernel_spmd

BF16 = mybir.dt.bfloat16
F32 = mybir.dt.float32
NP_BF16 = ml_dtypes.bfloat16

D = 8          # cores
T = 4096       # inner (contraction) dim = B*N
KT = T // 128  # 32 k-tiles
CA = 2048      # C/2 channels
MLOC = CA // D  # 256 output rows per core
O = 12288      # fc rows
OLOC = O // D   # 1536 fc rows per core
OT = OLOC // 128  # 12 o-tiles per core
C = 4096

_cache = {}


def _new_nc():
    return bacc.Bacc("TRN2", target_bir_lowering=False, debug=False, num_devices=D)


def _build_phase1():
    """Per-core: partial fc column-sum + sharded xa/xb column sq-norms."""
    nc = _new_nc()
    fc_d = nc.dram_tensor("fc", [128, OT, C], BF16, kind="ExternalInput").ap()
    xas_d = nc.dram_tensor("xas", [128, KT, MLOC], BF16, kind="ExternalInput").ap()
    xbs_d = nc.dram_tensor("xbs", [128, KT, MLOC], BF16, kind="ExternalInput").ap()
    wpart_d = nc.dram_tensor("wpart", [1, C], F32, kind="ExternalOutput").ap()
    nasl_d = nc.dram_tensor("nasl", [1, MLOC], F32, kind="ExternalOutput").ap()
    nbsl_d = nc.dram_tensor("nbsl", [1, MLOC], F32, kind="ExternalOutput").ap()

    with tile.TileContext(nc) as tc:
        with (
            tc.tile_pool(name="fcp", bufs=1) as fcp,
            tc.tile_pool(name="xp", bufs=1) as xp,
            tc.tile_pool(name="small", bufs=1) as small,
            tc.tile_pool(name="stage", bufs=2) as stage,
            tc.tile_pool(name="psw", bufs=4, space="PSUM") as psw,
            tc.tile_pool(name="psn", bufs=2, space="PSUM") as psn,
        ):
            ones = small.tile([128, 1], BF16)
            nc.vector.memset(ones[:], 1.0)

            # chunked fc load: one tile per o-tile so matmuls start early
            ft = []
            for ot in range(OT):
                f = fcp.tile([128, C], BF16, tag=f"fc{ot}")
                nc.sync.dma_start(f[:], fc_d[:, ot, :])
                ft.append(f)

            xas_sb = xp.tile([128, KT, MLOC], BF16)
            nc.sync.dma_start(xas_sb[:], xas_d[:])
            xbs_sb = xp.tile([128, KT, MLOC], BF16)
            nc.sync.dma_start(xbs_sb[:], xbs_d[:])

            # partial fc column sums; 4 psum banks per half
            wsb = stage.tile([1, C], F32)
            for half in range(2):
                pss = [psw.tile([1, 512], F32, name=f"psw{half}_{i}", tag="psw")
                       for i in range(4)]
                for ot in range(OT):
                    for ci, ps in enumerate(pss):
                        ch = half * 4 + ci
                        nc.tensor.matmul(
                            ps[:], ones[:],
                            ft[ot][:, ch * 512:(ch + 1) * 512],
                            start=(ot == 0), stop=(ot == OT - 1),
                        )
                for ci, ps in enumerate(pss):
                    ch = half * 4 + ci
                    nc.vector.tensor_copy(wsb[:, ch * 512:(ch + 1) * 512], ps[:])
            nc.sync.dma_start(wpart_d[:], wsb[:])

            # na / nb slices: square on ScalarE, column-sum over 32 k-tiles
            for (xsb, out_d, nm) in ((xas_sb, nasl_d, "a"), (xbs_sb, nbsl_d, "b")):
                x2 = xp.tile([128, KT, MLOC], BF16, name=f"x2{nm}")
                nc.scalar.square(x2[:], xsb[:])
                ps = psn.tile([1, MLOC], F32, name=f"psn{nm}", tag="psn")
                for kt in range(KT):
                    nc.tensor.matmul(
                        ps[:], ones[:], x2[:, kt, :],
                        start=(kt == 0), stop=(kt == KT - 1),
                    )
                st = stage.tile([1, MLOC], F32, name=f"st{nm}", tag="st")
                nc.vector.tensor_copy(st[:], ps[:])
                nc.sync.dma_start(out_d[:], st[:])

    nc.compile()
    return nc


def _build_phase2():
    """Per-core: 256 rows of scores = (wa+wb)^2 * relu(na+nb-2ab)."""
    nc = _new_nc()
    xasc_d = nc.dram_tensor("xasc", [128, KT, MLOC], BF16, kind="ExternalInput").ap()
    xbr_d = nc.dram_tensor("xbr", [128, KT, CA], BF16, kind="ExternalInput").ap()
    # packed per-partition vectors: [:, 0:2]=wa (per m-tile), [:, 2:4]=na
    pv_d = nc.dram_tensor("pv", [128, 4], F32, kind="ExternalInput").ap()
    # packed free-axis vectors: [0, 0:CA]=wb, [0, CA:2CA]=nb
    fv_d = nc.dram_tensor("fv", [1, 2 * CA], F32, kind="ExternalInput").ap()
    out_d = nc.dram_tensor("scores", [MLOC, CA], F32, kind="ExternalOutput").ap()

    NJ = CA // 512  # 4 column chunks
    MT = MLOC // 128  # 2 m-tiles
    KG = 2           # k-tiles per xb DMA chunk

    with tile.TileContext(nc) as tc:
        with (
            tc.tile_pool(name="xap", bufs=1) as xap,
            tc.tile_pool(name="xbp", bufs=1) as xbp,
            tc.tile_pool(name="small", bufs=1) as small,
            tc.tile_pool(name="w2p", bufs=1) as w2p,
            tc.tile_pool(name="sqp", bufs=4) as sqp,
            tc.tile_pool(name="outp", bufs=4) as outp,
            tc.tile_pool(name="psmm", bufs=8, space="PSUM") as psmm,
        ):
            pv = small.tile([128, 4], F32)
            nc.sync.dma_start(pv[:], pv_d[:])
            # broadcast wb|nb across partitions via DMA
            bcv = small.tile([128, 2 * CA], F32)
            nc.sync.dma_start(bcv[:], fv_d.to_broadcast([128, 2 * CA]))

            xasc = xap.tile([128, KT, MLOC], BF16)
            nc.sync.dma_start(xasc[:], xasc_d[:])

            xbt = []
            for h in range(KT // KG):
                xb_t = xbp.tile([128, KG, CA], BF16, tag=f"xb{h}")
                nc.sync.dma_start(xb_t[:], xbr_d[:, h * KG:(h + 1) * KG, :])
                xbt.append(xb_t)

            # W2[m][p, j] = (wa[m*128+p] + wb[j])^2
            w2 = []
            for m in range(MT):
                w2m = w2p.tile([128, CA], F32, tag=f"w2_{m}")
                nc.scalar.activation(
                    w2m[:], bcv[:, 0:CA],
                    mybir.ActivationFunctionType.Square,
                    bias=pv[:, m:m + 1], scale=1.0,
                )
                w2.append(w2m)

            # main matmul: k-OUTER accumulation into all 8 psum banks
            ps = [[psmm.tile([128, 512], F32, name=f"ps{m}_{nj}", tag="ps")
                   for nj in range(NJ)] for m in range(MT)]
            for kt in range(KT):
                h, r = divmod(kt, KG)
                for m in range(MT):
                    for nj in range(NJ):
                        nc.tensor.matmul(
                            ps[m][nj][:],
                            xasc[:, kt, m * 128:(m + 1) * 128],
                            xbt[h][:, r, nj * 512:(nj + 1) * 512],
                            start=(kt == 0), stop=(kt == KT - 1),
                        )

            for m in range(MT):
                for nj in range(NJ):
                    # sq = ps + na[m-part] + nb[chunk]
                    sq = sqp.tile([128, 512], F32, name=f"sq{m}_{nj}", tag="sq")
                    nc.vector.scalar_tensor_tensor(
                        sq[:], ps[m][nj][:], pv[:, 2 + m:3 + m],
                        bcv[:, CA + nj * 512:CA + (nj + 1) * 512],
                        op0=mybir.AluOpType.add, op1=mybir.AluOpType.add,
                    )
                    # out = relu(sq) * W2
                    ot = outp.tile([128, 512], F32, name=f"ot{m}_{nj}", tag="ot")
                    nc.vector.scalar_tensor_tensor(
                        ot[:], sq[:], 0.0,
                        w2[m][:, nj * 512:(nj + 1) * 512],
                        op0=mybir.AluOpType.max, op1=mybir.AluOpType.mult,
                    )
                    nc.sync.dma_start(
                        out_d[m * 128:(m + 1) * 128, nj * 512:(nj + 1) * 512],
                        ot[:],
                    )

    nc.compile()
    return nc


def _p_major(a, np_dtype):
    """[n*128, cols] -> [128, n, cols] with tile index in the middle."""
    n = a.shape[0] // 128
    return np.ascontiguousarray(
        a.reshape(n, 128, a.shape[1]).transpose(1, 0, 2).astype(np_dtype)
    )


def kernel(x, fc_weight, _trace=False):
    x = np.asarray(x, dtype=np.float32)
    fc = np.asarray(fc_weight, dtype=np.float32)

    xf = x.reshape(T, C)
    xa = np.ascontiguousarray(xf[:, 0::2])   # [T, CA]
    xb = np.ascontiguousarray(xf[:, 1::2])

    xb_r = _p_major(xb, NP_BF16)             # [128, KT, CA]
    xa_s2 = (-2.0 * xa)

    # ---- launch 1 ----
    if "p1" not in _cache:
        _cache["p1"] = _build_phase1()
    nc1 = _cache["p1"]

    in_maps1 = []
    for d in range(D):
        sl = slice(d * MLOC, (d + 1) * MLOC)
        in_maps1.append({
            "fc": _p_major(fc[d * OLOC:(d + 1) * OLOC], NP_BF16),
            "xas": _p_major(xa[:, sl], NP_BF16),
            "xbs": _p_major(xb[:, sl], NP_BF16),
        })
    res1 = run_bass_kernel_spmd(nc1, in_maps1, core_ids=list(range(D)), trace=_trace)
    t1 = res1.exec_time_ns

    wsum = np.sum([res1.results[d]["wpart"][0] for d in range(D)], axis=0,
                  dtype=np.float32)                              # [C]
    na = np.concatenate([res1.results[d]["nasl"][0] for d in range(D)])  # [CA]
    nb = np.concatenate([res1.results[d]["nbsl"][0] for d in range(D)])
    wa = np.ascontiguousarray(wsum[0::2])
    wb = np.ascontiguousarray(wsum[1::2])

    # ---- launch 2 ----
    if "p2" not in _cache:
        _cache["p2"] = _build_phase2()
    nc2 = _cache["p2"]

    fv = np.concatenate([wb, nb]).reshape(1, 2 * CA).astype(np.float32)
    in_maps2 = []
    for d in range(D):
        sl = slice(d * MLOC, (d + 1) * MLOC)
        pv = np.stack([
            wa[sl].reshape(2, 128)[0], wa[sl].reshape(2, 128)[1],
            na[sl].reshape(2, 128)[0], na[sl].reshape(2, 128)[1],
        ], axis=1)  # [128, 4]: [:,m]=wa m-tile, [:,2+m]=na m-tile
        in_maps2.append({
            "xasc": _p_major(xa_s2[:, sl], NP_BF16),
            "xbr": xb_r,
            "pv": np.ascontiguousarray(pv).astype(np.float32),
            "fv": fv,
        })
    res2 = run_bass_kernel_spmd(nc2, in_maps2, core_ids=list(range(D)), trace=_trace)
    t2 = res2.exec_time_ns

    out = np.concatenate([res2.results[d]["scores"] for d in range(D)], axis=0)
    if _trace:
        kernel.last_times = (t1, t2)
    return out.astype(np.float32)


# revision 4
# speedup vs baseline: 1.2840x; 1.0943x over previous
"""Trainium2 Bass kernel for nn_CRModule (retrieval_knn).

reference:
    xf = x.reshape(4096, 4096); xa = xf[:, ::2]; xb = xf[:, 1::2]   # [T=4096, 2048]
    sq[i,j] = |xa[:,i]|^2 + |xb[:,j]|^2 - 2 * xa[:,i].xb[:,j]
    wsum = fc_weight.sum(0); wa = wsum[::2]; wb = wsum[1::2]
    scores[i,j] = ((wa[i] + wb[j]) * sqrt(max(sq,0)))**2
                = (wa[i] + wb[j])**2 * max(sq[i,j], 0)     # sqrt cancels

Strategy (8 NeuronCores, two SPMD launches):
  Launch 1 (cross-core reductions, host combines 24 KB):
    fc_weight row-sharded (1536 rows/core, columns pre-split [even|odd])
    -> partial column sums wpart = [wa_part | wb_part]; xb column norms
    sharded over channels (256/core) -> nbsl.
  Launch 2 (main, row-sharded output): each core owns 256 rows of scores;
    (-2a)^T b in bf16 on PE, k-OUTER accumulation into all 8 PSUM banks
    so matmuls chase the chunked xb DMA stream; na computed on-device
    from (-2a)^2 * 0.25; fused fp32 DVE epilogue in-place in PSUM.
"""

import numpy as np
import ml_dtypes

import concourse.bass as bass
import concourse.tile as tile
from concourse import bacc, mybir
from concourse.bass_utils import run_bass_kernel_spmd

BF16 = mybir.dt.bfloat16
F32 = mybir.dt.float32
NP_BF16 = ml_dtypes.bfloat16

D = 8          # cores
T = 4096       # inner (contraction) dim = B*N
KT = T // 128  # 32 k-tiles
CA = 2048      # C/2 channels
MLOC = CA // D  # 256 output rows per core
O = 12288      # fc rows
OLOC = O // D   # 1536 fc rows per core
OT = OLOC // 128  # 12 o-tiles per core
C = 4096

_cache = {}


def _new_nc():
    return bacc.Bacc("TRN2", target_bir_lowering=False, debug=False, num_devices=D)


def _build_phase1():
    """Per-core: partial fc column-sum (cols pre-split [even|odd]) +
    sharded xb column sq-norms."""
    nc = _new_nc()
    fc_d = nc.dram_tensor("fc", [128, OT, C], BF16, kind="ExternalInput").ap()
    xbs_d = nc.dram_tensor("xbs", [128, KT, MLOC], BF16, kind="ExternalInput").ap()
    wpart_d = nc.dram_tensor("wpart", [1, C], F32, kind="ExternalOutput").ap()
    nbsl_d = nc.dram_tensor("nbsl", [1, MLOC], F32, kind="ExternalOutput").ap()

    with tile.TileContext(nc) as tc:
        with (
            tc.tile_pool(name="fcp", bufs=1) as fcp,
            tc.tile_pool(name="xp", bufs=1) as xp,
            tc.tile_pool(name="small", bufs=1) as small,
            tc.tile_pool(name="stage", bufs=2) as stage,
            tc.tile_pool(name="psw", bufs=4, space="PSUM") as psw,
            tc.tile_pool(name="psn", bufs=1, space="PSUM") as psn,
        ):
            ones = small.tile([128, 1], BF16)
            nc.vector.memset(ones[:], 1.0)

            # xb slice first (small), then chunked fc load
            xbs_sb = xp.tile([128, KT, MLOC], BF16)
            nc.sync.dma_start(xbs_sb[:], xbs_d[:])
            ft = []
            for ot in range(OT):
                f = fcp.tile([128, C], BF16, tag=f"fc{ot}")
                nc.sync.dma_start(f[:], fc_d[:, ot, :])
                ft.append(f)

            # nb slice: square on ScalarE, column-sum over 32 k-tiles
            x2 = xp.tile([128, KT, MLOC], BF16)
            nc.scalar.square(x2[:], xbs_sb[:])
            psb = psn.tile([1, MLOC], F32)
            for kt in range(KT):
                nc.tensor.matmul(
                    psb[:], ones[:], x2[:, kt, :],
                    start=(kt == 0), stop=(kt == KT - 1),
                )
            st = stage.tile([1, MLOC], F32)
            nc.vector.tensor_copy(st[:], psb[:])
            nc.sync.dma_start(nbsl_d[:], st[:])

            # partial fc column sums; 4 psum banks per half
            wsb = stage.tile([1, C], F32)
            for half in range(2):
                pss = [psw.tile([1, 512], F32, name=f"psw{half}_{i}", tag="psw")
                       for i in range(4)]
                for ot in range(OT):
                    for ci, ps in enumerate(pss):
                        ch = half * 4 + ci
                        nc.tensor.matmul(
                            ps[:], ones[:],
                            ft[ot][:, ch * 512:(ch + 1) * 512],
                            start=(ot == 0), stop=(ot == OT - 1),
                        )
                for ci, ps in enumerate(pss):
                    ch = half * 4 + ci
                    nc.vector.tensor_copy(wsb[:, ch * 512:(ch + 1) * 512], ps[:])
            nc.sync.dma_start(wpart_d[:], wsb[:])

    nc.compile()
    return nc


def _build_phase2():
    """Per-core: 256 rows of scores = (wa+wb)^2 * relu(na+nb-2ab)."""
    nc = _new_nc()
    xasc_d = nc.dram_tensor("xasc", [128, KT, MLOC], BF16, kind="ExternalInput").ap()
    xbr_d = nc.dram_tensor("xbr", [128, KT, CA], BF16, kind="ExternalInput").ap()
    # wa per-partition per m-tile
    pv_d = nc.dram_tensor("pv", [128, 2], F32, kind="ExternalInput").ap()
    # packed free-axis vectors: [0, 0:CA]=wb, [0, CA:2CA]=nb
    fv_d = nc.dram_tensor("fv", [1, 2 * CA], F32, kind="ExternalInput").ap()
    out_d = nc.dram_tensor("scores", [MLOC, CA], F32, kind="ExternalOutput").ap()
    na_dram = nc.dram_tensor("na_tmp", [1, MLOC], F32).ap()

    NJ = CA // 512   # 4 column chunks
    MT = MLOC // 128  # 2 m-tiles
    KG = 2           # k-tiles per xb DMA chunk
    XG = 8           # k-tiles per xa DMA chunk

    with tile.TileContext(nc) as tc:
        with (
            tc.tile_pool(name="xap", bufs=1) as xap,
            tc.tile_pool(name="xbp", bufs=1) as xbp,
            tc.tile_pool(name="small", bufs=1) as small,
            tc.tile_pool(name="w2p", bufs=1) as w2p,
            tc.tile_pool(name="x2p", bufs=2) as x2p,
            tc.tile_pool(name="outp", bufs=2) as outp,
        ):
            # ---- input streams (emission order = DMA issue order) ----
            xac = []
            for g in range(KT // XG):
                x_t = xap.tile([128, XG, MLOC], BF16, tag=f"xa{g}")
                nc.sync.dma_start(x_t[:], xasc_d[:, g * XG:(g + 1) * XG, :])
                xac.append(x_t)
            xbt = []
            for h in range(KT // KG):
                xb_t = xbp.tile([128, KG, CA], BF16, tag=f"xb{h}")
                nc.sync.dma_start(xb_t[:], xbr_d[:, h * KG:(h + 1) * KG, :])
                xbt.append(xb_t)

            quarter = small.tile([128, 1], BF16)
            nc.vector.memset(quarter[:], 0.25)

            # ---- na from (-2a)^2 * 0.25, then DRAM roundtrip to [128,2] ----
            with tc.tile_pool(name="psna", bufs=1, space="PSUM") as psna:
                psa = psna.tile([1, MLOC], F32)
                for g in range(KT // XG):
                    x2 = x2p.tile([128, XG, MLOC], BF16, tag="x2")
                    nc.scalar.square(x2[:], xac[g][:])
                    for s in range(XG):
                        kt = g * XG + s
                        nc.tensor.matmul(
                            psa[:], quarter[:], x2[:, s, :],
                            start=(kt == 0), stop=(kt == KT - 1),
                        )
                nast = small.tile([1, MLOC], F32)
                nc.vector.tensor_copy(nast[:], psa[:])
                nc.sync.dma_start(na_dram[:], nast[:])
            nav = small.tile([128, MT], F32)
            nc.sync.dma_start(
                nav[:],
                bass.AP(tensor=na_dram.tensor, offset=0, ap=[[1, 128], [128, MT]]),
            )

            # ---- main matmul: k-OUTER accumulation, 2 x 4-bank psum tiles ----
            with tc.tile_pool(name="psmm", bufs=2, space="PSUM") as psmm:
                ps = [psmm.tile([128, NJ, 512], F32, name=f"ps{m}", tag="ps")
                      for m in range(MT)]
                for kt in range(KT):
                    h, r = divmod(kt, KG)
                    g, s = divmod(kt, XG)
                    for m in range(MT):
                        for nj in range(NJ):
                            nc.tensor.matmul(
                                ps[m][:, nj, :],
                                xac[g][:, s, m * 128:(m + 1) * 128],
                                xbt[h][:, r, nj * 512:(nj + 1) * 512],
                                start=(kt == 0), stop=(kt == KT - 1),
                            )

                # ---- epilogue vectors (issued late; DMA overlaps MM stream) ----
                pv = small.tile([128, 2], F32)
                nc.sync.dma_start(pv[:], pv_d[:])
                wb_bc = small.tile([128, CA], F32)
                nc.sync.dma_start(wb_bc[:], fv_d[0:1, 0:CA].to_broadcast([128, CA]))
                nb_bc = small.tile([128, CA], F32)
                nc.sync.dma_start(nb_bc[:], fv_d[0:1, CA:2 * CA].to_broadcast([128, CA]))
                w2 = []
                for m in range(MT):
                    w2m = w2p.tile([128, CA], F32, tag=f"w2_{m}")
                    nc.scalar.activation(
                        w2m[:], wb_bc[:],
                        mybir.ActivationFunctionType.Square,
                        bias=pv[:, m:m + 1], scale=1.0,
                    )
                    w2.append(w2m)

                # ---- epilogue: sq in-place in psum, scale, store ----
                for m in range(MT):
                    pflat = ps[m].rearrange("p a b -> p (a b)")
                    nc.vector.scalar_tensor_tensor(
                        pflat, pflat, nav[:, m:m + 1], nb_bc[:],
                        op0=mybir.AluOpType.add, op1=mybir.AluOpType.add,
                    )
                    ot = outp.tile([128, CA], F32, tag="ot")
                    nc.vector.scalar_tensor_tensor(
                        ot[:], pflat, 0.0, w2[m][:],
                        op0=mybir.AluOpType.max, op1=mybir.AluOpType.mult,
                    )
                    nc.sync.dma_start(out_d[m * 128:(m + 1) * 128, :], ot[:])

    nc.compile()
    return nc


def _p_major(a, np_dtype):
    """[n*128, cols] -> [128, n, cols] with tile index in the middle."""
    n = a.shape[0] // 128
    return np.ascontiguousarray(
        a.reshape(n, 128, a.shape[1]).transpose(1, 0, 2).astype(np_dtype)
    )


def kernel(x, fc_weight, _trace=False):
    x = np.asarray(x, dtype=np.float32)
    fc = np.asarray(fc_weight, dtype=np.float32)

    xf = x.reshape(T, C)
    xa = np.ascontiguousarray(xf[:, 0::2])   # [T, CA]
    xb = np.ascontiguousarray(xf[:, 1::2])
    # deinterleave fc columns: [even | odd] so wpart = [wa_part | wb_part]
    fc_r = np.concatenate([fc[:, 0::2], fc[:, 1::2]], axis=1)

    xb_r = _p_major(xb, NP_BF16)             # [128, KT, CA]
    xa_s2 = -2.0 * xa

    # ---- launch 1 ----
    if "p1" not in _cache:
        _cache["p1"] = _build_phase1()
    nc1 = _cache["p1"]

    in_maps1 = []
    for d in range(D):
        sl = slice(d * MLOC, (d + 1) * MLOC)
        in_maps1.append({
            "fc": _p_major(fc_r[d * OLOC:(d + 1) * OLOC], NP_BF16),
            "xbs": _p_major(xb[:, sl], NP_BF16),
        })
    res1 = run_bass_kernel_spmd(nc1, in_maps1, core_ids=list(range(D)), trace=_trace)
    t1 = res1.exec_time_ns

    wsum = np.sum([res1.results[d]["wpart"][0] for d in range(D)], axis=0,
                  dtype=np.float32)                              # [C] = [wa|wb]
    nb = np.concatenate([res1.results[d]["nbsl"][0] for d in range(D)])
    wa, wb = wsum[:CA], wsum[CA:]

    # ---- launch 2 ----
    if "p2" not in _cache:
        _cache["p2"] = _build_phase2()
    nc2 = _cache["p2"]

    fv = np.concatenate([wb, nb]).reshape(1, 2 * CA).astype(np.float32)
    in_maps2 = []
    for d in range(D):
        sl = slice(d * MLOC, (d + 1) * MLOC)
        in_maps2.append({
            "xasc": _p_major(xa_s2[:, sl], NP_BF16),
            "xbr": xb_r,
            "pv": np.ascontiguousarray(wa[sl].reshape(2, 128).T).astype(np.float32),
            "fv": fv,
        })
    res2 = run_bass_kernel_spmd(nc2, in_maps2, core_ids=list(range(D)), trace=_trace)
    t2 = res2.exec_time_ns

    out = np.concatenate([res2.results[d]["scores"] for d in range(D)], axis=0)
    if _trace:
        kernel.last_times = (t1, t2)
    return out.astype(np.float32)


# revision 6
# speedup vs baseline: 1.3015x; 1.0136x over previous
"""Trainium2 Bass kernel for nn_CRModule (retrieval_knn).

reference:
    xf = x.reshape(4096, 4096); xa = xf[:, ::2]; xb = xf[:, 1::2]   # [T=4096, 2048]
    sq[i,j] = |xa[:,i]|^2 + |xb[:,j]|^2 - 2 * xa[:,i].xb[:,j]
    wsum = fc_weight.sum(0); wa = wsum[::2]; wb = wsum[1::2]
    scores[i,j] = ((wa[i] + wb[j]) * sqrt(max(sq,0)))**2
                = (wa[i] + wb[j])**2 * max(sq[i,j], 0)     # sqrt cancels

Strategy (8 NeuronCores, two SPMD launches):
  Launch 1 (cross-core reductions, host combines 24 KB):
    fc_weight row-sharded (1536 rows/core, columns pre-split [even|odd])
    -> partial column sums wpart = [wa_part | wb_part]; xb column norms
    sharded over channels (256/core) -> nbsl.
  Launch 2 (main, row-sharded output): each core owns 256 rows of scores;
    (-2a)^T b in bf16 on PE, k-OUTER accumulation into all 8 PSUM banks
    so matmuls chase the chunked xb DMA stream; na computed on-device
    from (-2a)^2 * 0.25; fused fp32 DVE epilogue in-place in PSUM.
"""

import numpy as np
import ml_dtypes

import concourse.bass as bass
import concourse.tile as tile
from concourse import bacc, mybir
from concourse.bass_utils import run_bass_kernel_spmd

BF16 = mybir.dt.bfloat16
F32 = mybir.dt.float32
NP_BF16 = ml_dtypes.bfloat16
FP8 = mybir.dt.float8e4
NP_FP8 = ml_dtypes.float8_e4m3

D = 8          # cores
T = 4096       # inner (contraction) dim = B*N
KT = T // 128  # 32 k-tiles
CA = 2048      # C/2 channels
MLOC = CA // D  # 256 output rows per core
O = 12288      # fc rows
OLOC = O // D   # 1536 fc rows per core
OT = OLOC // 128  # 12 o-tiles per core
C = 4096

_cache = {}


def _new_nc():
    return bacc.Bacc("TRN2", target_bir_lowering=False, debug=False, num_devices=D)


def _build_phase1():
    """Per-core: partial fc column-sum (cols pre-split [even|odd]) +
    sharded xb column sq-norms."""
    nc = _new_nc()
    fc_d = nc.dram_tensor("fc", [128, OT, C], BF16, kind="ExternalInput").ap()
    xbs_d = nc.dram_tensor("xbs", [128, KT, MLOC], BF16, kind="ExternalInput").ap()
    wpart_d = nc.dram_tensor("wpart", [1, C], F32, kind="ExternalOutput").ap()
    nbsl_d = nc.dram_tensor("nbsl", [1, MLOC], F32, kind="ExternalOutput").ap()

    with tile.TileContext(nc) as tc:
        with (
            tc.tile_pool(name="fcp", bufs=1) as fcp,
            tc.tile_pool(name="xp", bufs=1) as xp,
            tc.tile_pool(name="small", bufs=1) as small,
            tc.tile_pool(name="stage", bufs=2) as stage,
            tc.tile_pool(name="psw", bufs=4, space="PSUM") as psw,
            tc.tile_pool(name="psn", bufs=1, space="PSUM") as psn,
        ):
            ones = small.tile([128, 1], BF16)
            nc.vector.memset(ones[:], 1.0)

            # xb slice first (small), then chunked fc load
            xbs_sb = xp.tile([128, KT, MLOC], BF16)
            nc.sync.dma_start(xbs_sb[:], xbs_d[:])
            ft = []
            for ot in range(OT):
                f = fcp.tile([128, C], BF16, tag=f"fc{ot}")
                nc.sync.dma_start(f[:], fc_d[:, ot, :])
                ft.append(f)

            # nb slice: square on ScalarE, column-sum over 32 k-tiles
            x2 = xp.tile([128, KT, MLOC], BF16)
            nc.scalar.square(x2[:], xbs_sb[:])
            psb = psn.tile([1, MLOC], F32)
            for kt in range(KT):
                nc.tensor.matmul(
                    psb[:], ones[:], x2[:, kt, :],
                    start=(kt == 0), stop=(kt == KT - 1),
                )
            st = stage.tile([1, MLOC], F32)
            nc.vector.tensor_copy(st[:], psb[:])
            nc.sync.dma_start(nbsl_d[:], st[:])

            # partial fc column sums; 4 psum banks per half
            wsb = stage.tile([1, C], F32)
            for half in range(2):
                pss = [psw.tile([1, 512], F32, name=f"psw{half}_{i}", tag="psw")
                       for i in range(4)]
                for ot in range(OT):
                    for ci, ps in enumerate(pss):
                        ch = half * 4 + ci
                        nc.tensor.matmul(
                            ps[:], ones[:],
                            ft[ot][:, ch * 512:(ch + 1) * 512],
                            start=(ot == 0), stop=(ot == OT - 1),
                        )
                for ci, ps in enumerate(pss):
                    ch = half * 4 + ci
                    nc.vector.tensor_copy(wsb[:, ch * 512:(ch + 1) * 512], ps[:])
            nc.sync.dma_start(wpart_d[:], wsb[:])

    nc.compile()
    return nc


def _build_phase2():
    """Per-core: 256 rows of scores = (wa+wb)^2 * relu(na+nb-2ab)."""
    nc = _new_nc()
    xasc_d = nc.dram_tensor("xasc", [128, KT, MLOC], FP8, kind="ExternalInput").ap()
    xbr_d = nc.dram_tensor("xbr", [128, KT, CA], FP8, kind="ExternalInput").ap()
    # wa per-partition per m-tile
    pv_d = nc.dram_tensor("pv", [128, 2], F32, kind="ExternalInput").ap()
    # packed free-axis vectors: [0, 0:CA]=wb, [0, CA:2CA]=nb
    fv_d = nc.dram_tensor("fv", [1, 2 * CA], F32, kind="ExternalInput").ap()
    out_d = nc.dram_tensor("scores", [MLOC, CA], F32, kind="ExternalOutput").ap()
    na_dram = nc.dram_tensor("na_tmp", [1, MLOC], F32).ap()

    NJ = CA // 512   # 4 column chunks
    MT = MLOC // 128  # 2 m-tiles
    KG = 2           # k-tiles per xb DMA chunk
    XG = 8           # k-tiles per xa DMA chunk

    with tile.TileContext(nc) as tc:
        with (
            tc.tile_pool(name="xap", bufs=1) as xap,
            tc.tile_pool(name="xbp", bufs=1) as xbp,
            tc.tile_pool(name="small", bufs=1) as small,
            tc.tile_pool(name="w2p", bufs=1) as w2p,
            tc.tile_pool(name="x2p", bufs=2) as x2p,
            tc.tile_pool(name="outp", bufs=2) as outp,
        ):
            # ---- input streams (emission order = DMA issue order) ----
            xac = []
            for g in range(KT // XG):
                x_t = xap.tile([128, XG, MLOC], FP8, tag=f"xa{g}")
                nc.sync.dma_start(x_t[:], xasc_d[:, g * XG:(g + 1) * XG, :])
                xac.append(x_t)
            xbt = []
            for h in range(KT // KG):
                xb_t = xbp.tile([128, KG, CA], FP8, tag=f"xb{h}")
                nc.sync.dma_start(xb_t[:], xbr_d[:, h * KG:(h + 1) * KG, :])
                xbt.append(xb_t)

            quarter = small.tile([128, 1], BF16)
            nc.vector.memset(quarter[:], 0.25)

            # ---- na from (-2a)^2 * 0.25, then DRAM roundtrip to [128,2] ----
            with tc.tile_pool(name="psna", bufs=1, space="PSUM") as psna:
                psa = psna.tile([1, MLOC], F32)
                for g in range(KT // XG):
                    x2 = x2p.tile([128, XG, MLOC], BF16, tag="x2")
                    nc.scalar.square(x2[:], xac[g][:])
                    for s in range(XG):
                        kt = g * XG + s
                        nc.tensor.matmul(
                            psa[:], quarter[:], x2[:, s, :],
                            start=(kt == 0), stop=(kt == KT - 1),
                        )
                nast = small.tile([1, MLOC], F32)
                nc.vector.tensor_copy(nast[:], psa[:])
                nc.sync.dma_start(na_dram[:], nast[:])
            nav = small.tile([128, MT], F32)
            nc.sync.dma_start(
                nav[:],
                bass.AP(tensor=na_dram.tensor, offset=0, ap=[[1, 128], [128, MT]]),
            )

            # ---- main matmul: k-OUTER accumulation, 2 x 4-bank psum tiles ----
            with tc.tile_pool(name="psmm", bufs=2, space="PSUM") as psmm:
                ps = [psmm.tile([128, NJ, 512], F32, name=f"ps{m}", tag="ps")
                      for m in range(MT)]
                for kt in range(KT):
                    h, r = divmod(kt, KG)
                    g, s = divmod(kt, XG)
                    for m in range(MT):
                        for nj in range(NJ):
                            nc.tensor.matmul(
                                ps[m][:, nj, :],
                                xac[g][:, s, m * 128:(m + 1) * 128],
                                xbt[h][:, r, nj * 512:(nj + 1) * 512],
                                start=(kt == 0), stop=(kt == KT - 1),
                            )

                # ---- epilogue vectors (issued late; DMA overlaps MM stream) ----
                pv = small.tile([128, 2], F32)
                nc.sync.dma_start(pv[:], pv_d[:])
                wb_bc = small.tile([128, CA], F32)
                nc.sync.dma_start(wb_bc[:], fv_d[0:1, 0:CA].to_broadcast([128, CA]))
                nb_bc = small.tile([128, CA], F32)
                nc.sync.dma_start(nb_bc[:], fv_d[0:1, CA:2 * CA].to_broadcast([128, CA]))
                w2 = []
                for m in range(MT):
                    w2m = w2p.tile([128, CA], F32, tag=f"w2_{m}")
                    nc.scalar.activation(
                        w2m[:], wb_bc[:],
                        mybir.ActivationFunctionType.Square,
                        bias=pv[:, m:m + 1], scale=1.0,
                    )
                    w2.append(w2m)

                # ---- epilogue: sq in-place in psum, scale, store ----
                for m in range(MT):
                    pflat = ps[m].rearrange("p a b -> p (a b)")
                    nc.vector.scalar_tensor_tensor(
                        pflat, pflat, nav[:, m:m + 1], nb_bc[:],
                        op0=mybir.AluOpType.add, op1=mybir.AluOpType.add,
                    )
                    ot = outp.tile([128, CA], F32, tag="ot")
                    nc.vector.scalar_tensor_tensor(
                        ot[:], pflat, 0.0, w2[m][:],
                        op0=mybir.AluOpType.max, op1=mybir.AluOpType.mult,
                    )
                    nc.sync.dma_start(out_d[m * 128:(m + 1) * 128, :], ot[:])

    nc.compile()
    return nc


def _p_major(a, np_dtype):
    """[n*128, cols] -> [128, n, cols] with tile index in the middle."""
    n = a.shape[0] // 128
    return np.ascontiguousarray(
        a.reshape(n, 128, a.shape[1]).transpose(1, 0, 2).astype(np_dtype)
    )


def kernel(x, fc_weight, _trace=False):
    x = np.asarray(x, dtype=np.float32)
    fc = np.asarray(fc_weight, dtype=np.float32)

    xf = x.reshape(T, C)
    xa = np.ascontiguousarray(xf[:, 0::2])   # [T, CA]
    xb = np.ascontiguousarray(xf[:, 1::2])
    # deinterleave fc columns: [even | odd] so wpart = [wa_part | wb_part]
    fc_r = np.concatenate([fc[:, 0::2], fc[:, 1::2]], axis=1)

    xb_r = _p_major(xb, NP_FP8)              # [128, KT, CA]
    xa_s2 = -2.0 * xa

    # ---- launch 1 ----
    if "p1" not in _cache:
        _cache["p1"] = _build_phase1()
    nc1 = _cache["p1"]

    in_maps1 = []
    for d in range(D):
        sl = slice(d * MLOC, (d + 1) * MLOC)
        in_maps1.append({
            "fc": _p_major(fc_r[d * OLOC:(d + 1) * OLOC], NP_BF16),
            "xbs": _p_major(xb[:, sl], NP_BF16),
        })
    res1 = run_bass_kernel_spmd(nc1, in_maps1, core_ids=list(range(D)), trace=_trace)
    t1 = res1.exec_time_ns

    wsum = np.sum([res1.results[d]["wpart"][0] for d in range(D)], axis=0,
                  dtype=np.float32)                              # [C] = [wa|wb]
    nb = np.concatenate([res1.results[d]["nbsl"][0] for d in range(D)])
    wa, wb = wsum[:CA], wsum[CA:]

    # ---- launch 2 ----
    if "p2" not in _cache:
        _cache["p2"] = _build_phase2()
    nc2 = _cache["p2"]

    fv = np.concatenate([wb, nb]).reshape(1, 2 * CA).astype(np.float32)
    in_maps2 = []
    for d in range(D):
        sl = slice(d * MLOC, (d + 1) * MLOC)
        in_maps2.append({
            "xasc": _p_major(xa_s2[:, sl], NP_FP8),
            "xbr": xb_r,
            "pv": np.ascontiguousarray(wa[sl].reshape(2, 128).T).astype(np.float32),
            "fv": fv,
        })
    res2 = run_bass_kernel_spmd(nc2, in_maps2, core_ids=list(range(D)), trace=_trace)
    t2 = res2.exec_time_ns

    out = np.concatenate([res2.results[d]["scores"] for d in range(D)], axis=0)
    if _trace:
        kernel.last_times = (t1, t2)
    return out.astype(np.float32)
